# revision 1
# baseline (speedup 1.0000x reference)
"""ChainCRF NLL kernel for Trainium2 (8 NeuronCores, pure data parallel over B).

Algorithm (per core, BL=16 sequences):
  Phase A: feats = hidden @ W.T + b, computed as featsT [52, t] tiles via
    PE transpose of hidden tiles + bf16 matmul against host-transposed W.
    exp(featsT) lands in a per-chunk M buffer [54, 128*16] (t-major columns);
    raw featsT feeds the gold-emission dot against a host one-hot (Pool engine).
  Phase B: exp-domain linear recursion
       Ehat_{t+1} = expFeat_t * (TrAug @ Ehat_t)
    with TrAug carrying: exp(trans)/C transition block, exp(trans[END,:])/C
    capture column (Z row), A accumulator column (A' = A + Z), and a 1/C ones
    column producing Shat for periodic rescaling (every R steps, Ehat rows
    only). The delta row of M (host data) selects Z at t == len[b]-1.
  Host: nll = [log(A+Z) + (v+1)*logC + sum of event logS before v] - gold.
"""

import numpy as np
import ml_dtypes

import concourse.bass as bass
import concourse.bacc as bacc
import concourse.tile as tile
from concourse import mybir
from concourse.bass_utils import run_bass_kernel_spmd

B, T, H, K = 128, 1024, 512, 52
ROOT, END = 0, 1
NCORE = 8
BL = B // NCORE          # 16 sequences per core
NS = K + 2               # state rows: 52 Ehat + Z + A
NO = 65                  # out rows: 52 U + Z + A + pad, Shat at partition 64
R = 32                   # rescale period
NEV = T // R             # 32 events
LOGC = 4.9               # constant per-step rescale (exp-domain drift removal)

F32 = mybir.dt.float32
BF16 = mybir.dt.bfloat16

_NC_CACHE = {}


def build_bass():
    nc = bacc.Bacc(None)
    hid = nc.dram_tensor("hid", [BL, T, H], F32, kind="ExternalInput")
    wT = nc.dram_tensor("wT", [H, K], BF16, kind="ExternalInput")
    bvec = nc.dram_tensor("bvec", [K, 1], F32, kind="ExternalInput")
    trAug = nc.dram_tensor("trAug", [NS, NO], F32, kind="ExternalInput")
    s0 = nc.dram_tensor("s0", [NS, BL], F32, kind="ExternalInput")
    mtail = nc.dram_tensor("mtail", [2, T * BL], F32, kind="ExternalInput")
    onehot = nc.dram_tensor("onehot", [BL, K, T], F32, kind="ExternalInput")
    ident = nc.dram_tensor("ident", [128, 128], F32, kind="ExternalInput")
    ones_r = nc.dram_tensor("ones_r", [1, K], F32, kind="ExternalInput")
    ones_c = nc.dram_tensor("ones_c", [K, 1], F32, kind="ExternalInput")

    sfinal = nc.dram_tensor("sfinal", [NS, BL], F32, kind="ExternalOutput")
    scap_d = nc.dram_tensor("scap", [1, NEV * BL], F32, kind="ExternalOutput")
    emit_d = nc.dram_tensor("emit", [K, BL], F32, kind="ExternalOutput")

    NCHUNK = T // 128    # 8 time chunks of 128 steps

    with tile.TileContext(nc) as tc:
        with (
            tc.tile_pool(name="consts", bufs=1) as consts,
            tc.tile_pool(name="mbuf", bufs=1) as mbuf,
            tc.tile_pool(name="hids", bufs=3) as hids,
            tc.tile_pool(name="hts", bufs=3) as hts,
            tc.tile_pool(name="fr", bufs=3) as frp,
            tc.tile_pool(name="oh", bufs=3) as ohp,
            tc.tile_pool(name="prod", bufs=3) as prp,
            tc.tile_pool(name="red", bufs=3) as rdp,
            tc.tile_pool(name="state", bufs=3) as spool,
            tc.tile_pool(name="small", bufs=2) as smallp,
            tc.tile_pool(name="pt", bufs=2, space="PSUM") as ptp,
            tc.tile_pool(name="pf", bufs=2, space="PSUM") as pfp,
            tc.tile_pool(name="pr", bufs=2, space="PSUM") as prpsum,
            tc.tile_pool(name="pb", bufs=1, space="PSUM") as pbp,
        ):
            # ---- constants ----
            wT_sb = consts.tile([128, 4, K], BF16, tag="wT")
            nc.sync.dma_start(wT_sb, wT.rearrange("(c p) k -> p c k", p=128))
            trAug_sb = consts.tile([NS, NO], F32, tag="trAug")
            nc.sync.dma_start(trAug_sb, trAug[:, :])
            bias_sb = consts.tile([K, 1], F32, tag="bvec")
            nc.sync.dma_start(bias_sb, bvec[:, :])
            ident_sb = consts.tile([128, 128], F32, tag="ident")
            nc.sync.dma_start(ident_sb, ident[:, :])
            ones_r_sb = consts.tile([1, K], F32, tag="ones_r")
            nc.sync.dma_start(ones_r_sb, ones_r[:, :])
            ones_c_sb = consts.tile([K, 1], F32, tag="ones_c")
            nc.sync.dma_start(ones_c_sb, ones_c[:, :])
            scap_sb = consts.tile([1, NEV * BL], F32, tag="scap")
            prodaccs = []
            for b in range(BL):
                pa = consts.tile([K, 128], F32, tag=f"pacc{b}")
                nc.gpsimd.memset(pa, 0.0)
                prodaccs.append(pa)

            mchunks = []
            for c in range(NCHUNK):
                mc = mbuf.tile([NS, 128 * BL], F32, tag=f"m{c}")
                nc.sync.dma_start(
                    mc[K : K + 2, :], mtail[:, c * 128 * BL : (c + 1) * 128 * BL]
                )
                mchunks.append(mc)

            s_cur = spool.tile([NS, BL], F32, tag="state")
            nc.sync.dma_start(s_cur, s0[:, :])

            for c in range(NCHUNK):
                # ---- phase A for time chunk c: all BL sequences ----
                for b in range(BL):
                    hid_t = hids.tile([128, H], F32, tag="hid")
                    nc.sync.dma_start(hid_t, hid[b, c * 128 : (c + 1) * 128, :])
                    pt_t = ptp.tile([128, H], F32, tag="pt")
                    for ch in range(4):
                        nc.tensor.transpose(
                            pt_t[:, ch * 128 : (ch + 1) * 128],
                            hid_t[:, ch * 128 : (ch + 1) * 128],
                            ident_sb,
                        )
                    hT_t = hts.tile([128, H], BF16, tag="hT")
                    nc.scalar.copy(hT_t, pt_t)
                    pf_t = pfp.tile([K, 128], F32, tag="pf")
                    for ch in range(4):
                        nc.tensor.matmul(
                            pf_t,
                            wT_sb[:, ch, :],
                            hT_t[:, ch * 128 : (ch + 1) * 128],
                            start=(ch == 0),
                            stop=(ch == 3),
                        )
                    # exp(feats + b) into M rows 0:52 (columns strided by BL)
                    mview = mchunks[c][0:K, :].rearrange(
                        "p (t b) -> p t b", b=BL
                    )[:, :, b : b + 1]
                    nc.scalar.activation(
                        mview, pf_t, mybir.ActivationFunctionType.Exp,
                        bias=bias_sb, scale=1.0,
                    )
                    # raw feats + one-hot dot for the gold emission term
                    fraw_t = frp.tile([K, 128], F32, tag="fraw")
                    nc.scalar.activation(
                        fraw_t, pf_t, mybir.ActivationFunctionType.Identity,
                        bias=bias_sb, scale=1.0,
                    )
                    oh_t = ohp.tile([K, 128], F32, tag="oh")
                    nc.sync.dma_start(oh_t, onehot[b, :, c * 128 : (c + 1) * 128])
                    prod_t = prp.tile([K, 128], F32, tag="prod")
                    nc.gpsimd.tensor_mul(prod_t, fraw_t, oh_t)
                    nc.gpsimd.tensor_add(prodaccs[b], prodaccs[b], prod_t)

                # ---- phase B: recursion steps for chunk c ----
                for ti in range(128):
                    t = c * 128 + ti
                    p_t = prpsum.tile([NO, BL], F32, tag="pr")
                    nc.tensor.matmul(p_t, trAug_sb, s_cur, start=True, stop=True)
                    s_next = spool.tile([NS, BL], F32, tag="state")
                    nc.vector.tensor_mul(
                        s_next,
                        mchunks[c][:, ti * BL : (ti + 1) * BL],
                        p_t[0:NS, :],
                    )
                    if (t + 1) % R == 0:
                        e = (t + 1) // R - 1
                        srec = scap_sb[0:1, e * BL : (e + 1) * BL]
                        nc.vector.reciprocal(srec, p_t[NO - 1 : NO, :])
                        bc_t = pbp.tile([K, BL], F32, tag="pb")
                        nc.tensor.matmul(bc_t, ones_r_sb, srec, start=True, stop=True)
                        nc.vector.tensor_mul(s_next[0:K, :], s_next[0:K, :], bc_t)
                    s_cur = s_next

            # ---- outputs ----
            nc.sync.dma_start(sfinal[:, :], s_cur)
            nc.sync.dma_start(scap_d[:, :], scap_sb)
            emitred = smallp.tile([K, BL], F32, tag="em")
            for b in range(BL):
                nc.vector.tensor_reduce(
                    emitred[:, b : b + 1], prodaccs[b],
                    axis=mybir.AxisListType.X, op=mybir.AluOpType.add,
                )
            nc.sync.dma_start(emit_d[:, :], emitred)

    nc.compile()
    return nc


def kernel(hidden, W, b, log_transitions, tags, lengths):
    hidden = np.ascontiguousarray(hidden, dtype=np.float32)
    W = np.asarray(W, dtype=np.float32)
    b = np.asarray(b, dtype=np.float32)
    trans = np.asarray(log_transitions, dtype=np.float32)
    tags = np.asarray(tags, dtype=np.int32)
    lengths = np.asarray(lengths, dtype=np.int32)

    C = np.float64(np.exp(LOGC))
    expTr = np.exp(trans.astype(np.float64))
    trAug = np.zeros((NS, NO), dtype=np.float64)
    trAug[:K, :K] = expTr.T / C
    trAug[:K, K] = expTr[END, :] / C          # Z capture column
    trAug[K, K + 1] = 1.0                     # A' = A + Z
    trAug[K + 1, K + 1] = 1.0
    trAug[:K, NO - 1] = 1.0 / C               # Shat column (partition 64: quadrant-aligned)
    trAug = trAug.astype(np.float32)

    s0 = np.zeros((NS, BL), dtype=np.float32)
    s0[ROOT, :] = 1.0

    v = (lengths.astype(np.int64) - 1)        # capture step per sequence
    pos = np.arange(T)[None, :]
    maskT = pos < lengths[:, None]
    is_last = pos == (lengths[:, None] - 1)
    emask = (maskT & ~is_last)

    # one-hot [B, K, T] f32, masked to t <= len-2
    onehot = np.zeros((B, K, T), dtype=np.float32)
    bi, ti = np.nonzero(emask)
    onehot[bi, tags[bi, ti], ti] = 1.0

    wT_np = np.ascontiguousarray(W.T).astype(ml_dtypes.bfloat16)
    bvec = np.ascontiguousarray(b.reshape(K, 1))
    ident = np.eye(128, dtype=np.float32)
    ones_r = np.ones((1, K), dtype=np.float32)
    ones_c = np.ones((K, 1), dtype=np.float32)

    in_maps = []
    for core in range(NCORE):
        bs = slice(core * BL, (core + 1) * BL)
        v_c = v[bs]
        mtail = np.zeros((2, T * BL), dtype=np.float32)
        tt = np.arange(T)
        delta = (tt[:, None] == v_c[None, :]).astype(np.float32)   # [T, BL]
        mtail[0, :] = delta.reshape(-1)
        mtail[1, :] = 1.0
        in_maps.append({
            "hid": np.ascontiguousarray(hidden[bs]),
            "wT": wT_np,
            "bvec": bvec,
            "trAug": trAug,
            "s0": s0,
            "mtail": mtail,
            "onehot": np.ascontiguousarray(onehot[bs]),
            "ident": ident,
            "ones_r": ones_r,
            "ones_c": ones_c,
        })

    key = "nc"
    if key not in _NC_CACHE:
        _NC_CACHE[key] = build_bass()
    nc = _NC_CACHE[key]

    res = run_bass_kernel_spmd(nc, in_maps, core_ids=list(range(NCORE)))
    outs = res.results

    # ---- host assembly ----
    nll = np.zeros(B, dtype=np.float64)
    ev_steps = R * np.arange(1, NEV + 1) - 1                      # [NEV]
    tags_ext = np.concatenate(
        [np.full((B, 1), ROOT, tags.dtype), tags], axis=1
    )
    tr_score = (trans[tags, tags_ext[:, :-1]].astype(np.float64) * maskT).sum(axis=1)

    for core in range(NCORE):
        bs = slice(core * BL, (core + 1) * BL)
        v_c = v[bs]
        sfin = outs[core]["sfinal"].astype(np.float64)
        scap = outs[core]["scap"].reshape(NEV, BL).astype(np.float64)
        emit = outs[core]["emit"].astype(np.float64).sum(axis=0)
        AZ = sfin[K] + sfin[K + 1]
        prefix_mask = ev_steps[:, None] < v_c[None, :]
        logS_prefix = (-np.log(scap) * prefix_mask).sum(axis=0)
        log_z = np.log(AZ) + (v_c + 1) * LOGC + logS_prefix
        nll[bs] = log_z - tr_score[bs] - emit

    return nll.astype(np.float32)



# revision 3
# speedup vs baseline: 5.4697x; 5.4697x over previous
"""ChainCRF NLL kernel for Trainium2 (8 NeuronCores, pure data parallel over B).

Transfer-optimized design (the axon tunnel at ~75 MB/s dominates the span):
  - hidden ships as fp8e4m3, host-pre-transposed to [H-chunk, t] layout and
    packed with the (x16-scaled) fp8 W into one DRAM tensor per core.
  - gold score (transitions + emissions) is computed exactly on host in f32.
  - device computes feats via fp8 matmul, exp(feats/16 + b) into a per-chunk
    M buffer, then runs the exp-domain linear recursion
        Ehat_{t+1} = expFeat_t * (TrAug @ Ehat_t)
    with TrAug carrying the exp(trans)/C block, a Z capture column (selected
    by the host-supplied delta row at t == len[b]-1), an A accumulator
    (A' = A + Z), and a 1/C ones column producing Shat for periodic rescale.
  - host: nll = [log(A+Z) + (v+1)*logC + sum of event logS before v] - gold.
"""

import numpy as np
import ml_dtypes

import concourse.bass as bass
import concourse.bacc as bacc
import concourse.tile as tile
from concourse import mybir
from concourse.bass_utils import run_bass_kernel_spmd

B, T, H, K = 128, 1024, 512, 52
ROOT, END = 0, 1
NCORE = 8
BL = B // NCORE          # 16 sequences per core
NS = K + 2               # state rows: 52 Ehat + Z + A
NO = 65                  # out rows: 52 U + Z + A + pad, Shat at partition 64
R = 32                   # rescale period
NEV = T // R             # 32 events
LOGC = 4.9               # constant per-step rescale (exp-domain drift removal)
WSCALE = 16.0            # fp8 range scaling for W; undone by activation scale

NCHUNK = T // 128        # 8 time chunks of 128 steps
HC = H // 128            # 4 H-chunks
HIDCOL = NCHUNK * HC * BL * 128   # 65536 fp8 cols of packed hidden
PACKCOL = HIDCOL + HC * K         # + 208 cols of packed wT

F32 = mybir.dt.float32
FP8 = mybir.dt.float8e4

_NC_CACHE = {}


def build_bass():
    nc = bacc.Bacc(None)
    hpack = nc.dram_tensor("hpack", [128, PACKCOL], FP8, kind="ExternalInput")
    cpack = nc.dram_tensor("cpack", [NS, NO + 1], F32, kind="ExternalInput")
    mdelta = nc.dram_tensor("mdelta", [NCHUNK, 128 * BL], F32, kind="ExternalInput")

    sfinal = nc.dram_tensor("sfinal", [NS, BL], F32, kind="ExternalOutput")
    scap_d = nc.dram_tensor("scap", [1, NEV * BL], F32, kind="ExternalOutput")

    with tile.TileContext(nc) as tc:
        with (
            tc.tile_pool(name="consts", bufs=1) as consts,
            tc.tile_pool(name="mbuf", bufs=1) as mbuf,
            tc.tile_pool(name="state", bufs=3) as spool,
            tc.tile_pool(name="pf", bufs=4, space="PSUM") as pfp,
            tc.tile_pool(name="pr", bufs=2, space="PSUM") as prpsum,
            tc.tile_pool(name="pb", bufs=1, space="PSUM") as pbp,
        ):
            # ---- constants / inputs resident in SBUF ----
            hid_sb = consts.tile([128, NCHUNK, HC, BL * 128], FP8, tag="hid")
            nc.sync.dma_start(
                hid_sb,
                hpack[:, 0:HIDCOL].rearrange(
                    "p (c h x) -> p c h x", c=NCHUNK, h=HC
                ),
            )
            wT_sb = consts.tile([128, HC, K], FP8, tag="wT")
            nc.sync.dma_start(
                wT_sb, hpack[:, HIDCOL:PACKCOL].rearrange("p (h k) -> p h k", h=HC)
            )
            trAug_sb = consts.tile([NS, NO], F32, tag="trAug")
            nc.sync.dma_start(trAug_sb, cpack[:, 0:NO])
            bias_sb = consts.tile([K, 1], F32, tag="bvec")
            nc.sync.dma_start(bias_sb, cpack[0:K, NO : NO + 1])
            ones_r_sb = consts.tile([1, K], F32, tag="ones_r")
            nc.gpsimd.memset(ones_r_sb, 1.0)
            scap_sb = consts.tile([1, NEV * BL], F32, tag="scap")

            mchunks = []
            for c in range(NCHUNK):
                mc = mbuf.tile([NS, 128 * BL], F32, tag=f"m{c}")
                nc.gpsimd.memset(mc, 1.0)
                nc.sync.dma_start(mc[K : K + 1, :], mdelta[c : c + 1, :])
                mchunks.append(mc)

            s_cur = spool.tile([NS, BL], F32, tag="state")
            nc.gpsimd.memset(s_cur, 0.0)
            nc.gpsimd.memset(s_cur[ROOT : ROOT + 1, :], 1.0)

            # ---- phase A: feats for all chunks ----
            for c in range(NCHUNK):
                for b in range(BL):
                    pf_t = pfp.tile([K, 128], F32, tag="pf")
                    for ch in range(HC):
                        nc.tensor.matmul(
                            pf_t,
                            wT_sb[:, ch, :],
                            hid_sb[:, c, ch, b * 128 : (b + 1) * 128],
                            start=(ch == 0),
                            stop=(ch == HC - 1),
                        )
                    # exp(feats/WSCALE + b) into M rows 0:52 (cols strided by BL)
                    mview = mchunks[c][0:K, :].rearrange(
                        "p (t b) -> p t b", b=BL
                    )[:, :, b : b + 1]
                    nc.scalar.activation(
                        mview, pf_t, mybir.ActivationFunctionType.Exp,
                        bias=bias_sb, scale=1.0 / WSCALE,
                    )

            # ---- phase B: the 1024-step recursion ----
            for t in range(T):
                c, ti = divmod(t, 128)
                p_t = prpsum.tile([NO, BL], F32, tag="pr")
                nc.tensor.matmul(p_t, trAug_sb, s_cur, start=True, stop=True)
                s_next = spool.tile([NS, BL], F32, tag="state")
                nc.vector.tensor_mul(
                    s_next,
                    mchunks[c][:, ti * BL : (ti + 1) * BL],
                    p_t[0:NS, :],
                )
                if (t + 1) % R == 0:
                    e = (t + 1) // R - 1
                    srec = scap_sb[0:1, e * BL : (e + 1) * BL]
                    nc.vector.reciprocal(srec, p_t[NO - 1 : NO, :])
                    bc_t = pbp.tile([K, BL], F32, tag="pb")
                    nc.tensor.matmul(bc_t, ones_r_sb, srec, start=True, stop=True)
                    nc.vector.tensor_mul(s_next[0:K, :], s_next[0:K, :], bc_t)
                s_cur = s_next

            # ---- outputs ----
            nc.sync.dma_start(sfinal[:, :], s_cur)
            nc.sync.dma_start(scap_d[:, :], scap_sb)

    nc.compile()
    return nc


def kernel(hidden, W, b, log_transitions, tags, lengths):
    hidden = np.asarray(hidden, dtype=np.float32)
    W = np.asarray(W, dtype=np.float32)
    b = np.asarray(b, dtype=np.float32)
    trans = np.asarray(log_transitions, dtype=np.float32)
    tags = np.asarray(tags, dtype=np.int32)
    lengths = np.asarray(lengths, dtype=np.int32)

    C = np.float64(np.exp(LOGC))
    expTr = np.exp(trans.astype(np.float64))
    trAug = np.zeros((NS, NO), dtype=np.float64)
    trAug[:K, :K] = expTr.T / C
    trAug[:K, K] = expTr[END, :] / C          # Z capture column
    trAug[K, K + 1] = 1.0                     # A' = A + Z
    trAug[K + 1, K + 1] = 1.0
    trAug[:K, NO - 1] = 1.0 / C               # Shat column (partition 64)
    trAug = trAug.astype(np.float32)

    cpack = np.zeros((NS, NO + 1), dtype=np.float32)
    cpack[:, 0:NO] = trAug
    cpack[0:K, NO] = b

    v = (lengths.astype(np.int64) - 1)        # capture step per sequence
    pos = np.arange(T)[None, :]
    maskT = pos < lengths[:, None]
    is_last = pos == (lengths[:, None] - 1)
    emask = (maskT & ~is_last)

    # ---- fp8 packed, transposed hidden: [core, 128, PACKCOL] ----
    h8 = hidden.astype(ml_dtypes.float8_e4m3)
    # [core, b, c, x, ch, p] -> [core, p, c, ch, b, x]
    hperm = np.ascontiguousarray(
        h8.reshape(NCORE, BL, NCHUNK, 128, HC, 128).transpose(0, 5, 2, 4, 1, 3)
    ).reshape(NCORE, 128, HIDCOL)
    wT8 = np.ascontiguousarray(
        (W * WSCALE).astype(ml_dtypes.float8_e4m3)
        .T.reshape(HC, 128, K).transpose(1, 0, 2)
    ).reshape(128, HC * K)

    in_maps = []
    for core in range(NCORE):
        bs = slice(core * BL, (core + 1) * BL)
        v_c = v[bs]
        tt = np.arange(T)
        delta = (tt[:, None] == v_c[None, :]).astype(np.float32)   # [T, BL]
        hpack = np.concatenate([hperm[core], wT8], axis=1)
        in_maps.append({
            "hpack": hpack,
            "cpack": cpack,
            "mdelta": np.ascontiguousarray(delta.reshape(NCHUNK, 128 * BL)),
        })

    key = "nc"
    if key not in _NC_CACHE:
        _NC_CACHE[key] = build_bass()
    nc = _NC_CACHE[key]

    res = run_bass_kernel_spmd(nc, in_maps, core_ids=list(range(NCORE)))
    outs = res.results

    # ---- host gold score (exact f32): transitions + emissions ----
    tags_ext = np.concatenate(
        [np.full((B, 1), ROOT, tags.dtype), tags], axis=1
    )
    tr_score = (trans[tags, tags_ext[:, :-1]].astype(np.float64) * maskT).sum(axis=1)
    emit_score = np.zeros(B, dtype=np.float64)
    for core in range(NCORE):
        bs = slice(core * BL, (core + 1) * BL)
        Wg = W[tags[bs]]                                     # [BL, T, H]
        ef = np.einsum("bth,bth->bt", hidden[bs], Wg) + b[tags[bs]]
        emit_score[bs] = (ef.astype(np.float64) * emask[bs]).sum(axis=1)

    # ---- assemble nll ----
    nll = np.zeros(B, dtype=np.float64)
    ev_steps = R * np.arange(1, NEV + 1) - 1                 # [NEV]
    for core in range(NCORE):
        bs = slice(core * BL, (core + 1) * BL)
        v_c = v[bs]
        sfin = outs[core]["sfinal"].astype(np.float64)
        scap = outs[core]["scap"].reshape(NEV, BL).astype(np.float64)
        AZ = sfin[K] + sfin[K + 1]
        prefix_mask = ev_steps[:, None] < v_c[None, :]
        logS_prefix = (-np.log(scap) * prefix_mask).sum(axis=0)
        log_z = np.log(AZ) + (v_c + 1) * LOGC + logS_prefix
        nll[bs] = log_z - tr_score[bs] - emit_score[bs]

    return nll.astype(np.float32)


# revision 4
# speedup vs baseline: 7.4761x; 1.3668x over previous
"""ChainCRF NLL kernel for Trainium2 (8 NeuronCores, data parallel over B).

Transfer-optimized design (the axon tunnel at ~75 MB/s dominates the span):
  - hidden ships as fp8e4m3, host-pre-transposed to [H-chunk, t] layout and
    packed with the (x16-scaled) fp8 W into one DRAM tensor per core.
  - sequences are assigned to cores round-robin by descending-length rank, so
    all cores share one static per-slot chunk budget nb[b] =
    ceil(max-length-in-rank-group-b / 128); only those chunks ship. Columns
    of the M buffer beyond a sequence's budget stay at 1.0 — the recursion
    there decays geometrically and the periodic rescale renormalizes it, so
    the Z/A capture rows are unaffected.
  - gold score (transitions + emissions) is computed exactly on host in f32.
  - device computes feats via fp8 matmul, exp(feats/16 + b) into per-chunk
    M buffers, then runs the exp-domain linear recursion
        Ehat_{t+1} = expFeat_t * (TrAug @ Ehat_t)
    with TrAug carrying the exp(trans)/C block, a Z capture column (selected
    by the host-supplied delta row at t == len[b]-1), an A accumulator
    (A' = A + Z), and a 1/C ones column producing Shat for periodic rescale.
  - host: nll = [log(A+Z) + (v+1)*logC + sum of event logS before v] - gold.

The NEFF is specialized on the budget tuple nb (derived from lengths) and
cached per-process; a different length profile just triggers a recompile.
"""

import numpy as np
import ml_dtypes

import concourse.bass as bass
import concourse.bacc as bacc
import concourse.tile as tile
from concourse import mybir
from concourse.bass_utils import run_bass_kernel_spmd

B, T, H, K = 128, 1024, 512, 52
ROOT, END = 0, 1
NCORE = 8
BL = B // NCORE          # 16 sequences per core
NS = K + 2               # state rows: 52 Ehat + Z + A
NO = 65                  # out rows: 52 U + Z + A + pad, Shat at partition 64
R = 32                   # rescale period
NEV = T // R             # 32 events
LOGC = 4.9               # constant per-step rescale (exp-domain drift removal)
WSCALE = 16.0            # fp8 range scaling for W; undone by activation scale

NCHUNK = T // 128        # 8 time chunks of 128 steps
HC = H // 128            # 4 H-chunks

F32 = mybir.dt.float32
FP8 = mybir.dt.float8e4

_NC_CACHE = {}


def build_bass(nb):
    nslot = sum(nb)
    hidcol = nslot * HC * 128
    packcol = hidcol + HC * K

    nc = bacc.Bacc(None)
    hpack = nc.dram_tensor("hpack", [128, packcol], FP8, kind="ExternalInput")
    cpack = nc.dram_tensor("cpack", [NS, NO + 1], F32, kind="ExternalInput")
    mdelta = nc.dram_tensor("mdelta", [NCHUNK, 128 * BL], F32, kind="ExternalInput")

    sfinal = nc.dram_tensor("sfinal", [NS, BL], F32, kind="ExternalOutput")
    scap_d = nc.dram_tensor("scap", [1, NEV * BL], F32, kind="ExternalOutput")

    with tile.TileContext(nc) as tc:
        with (
            tc.tile_pool(name="consts", bufs=1) as consts,
            tc.tile_pool(name="mbuf", bufs=1) as mbuf,
            tc.tile_pool(name="state", bufs=3) as spool,
            tc.tile_pool(name="pf", bufs=4, space="PSUM") as pfp,
            tc.tile_pool(name="pr", bufs=2, space="PSUM") as prpsum,
            tc.tile_pool(name="pb", bufs=1, space="PSUM") as pbp,
        ):
            # ---- constants / inputs resident in SBUF ----
            hid_sb = consts.tile([128, nslot, HC, 128], FP8, tag="hid")
            nc.sync.dma_start(
                hid_sb,
                hpack[:, 0:hidcol].rearrange("p (s h x) -> p s h x", s=nslot, h=HC),
            )
            wT_sb = consts.tile([128, HC, K], FP8, tag="wT")
            nc.sync.dma_start(
                wT_sb, hpack[:, hidcol:packcol].rearrange("p (h k) -> p h k", h=HC)
            )
            trAug_sb = consts.tile([NS, NO], F32, tag="trAug")
            nc.sync.dma_start(trAug_sb, cpack[:, 0:NO])
            bias_sb = consts.tile([K, 1], F32, tag="bvec")
            nc.sync.dma_start(bias_sb, cpack[0:K, NO : NO + 1])
            ones_r_sb = consts.tile([1, K], F32, tag="ones_r")
            nc.gpsimd.memset(ones_r_sb, 1.0)
            scap_sb = consts.tile([1, NEV * BL], F32, tag="scap")

            mchunks = []
            for c in range(NCHUNK):
                mc = mbuf.tile([NS, 128 * BL], F32, tag=f"m{c}")
                nc.gpsimd.memset(mc, 1.0)
                nc.sync.dma_start(mc[K : K + 1, :], mdelta[c : c + 1, :])
                mchunks.append(mc)

            s_cur = spool.tile([NS, BL], F32, tag="state")
            nc.gpsimd.memset(s_cur, 0.0)
            nc.gpsimd.memset(s_cur[ROOT : ROOT + 1, :], 1.0)

            # ---- phase A: feats for all budgeted (slot, chunk) pairs ----
            s_i = 0
            for b in range(BL):
                for c in range(nb[b]):
                    pf_t = pfp.tile([K, 128], F32, tag="pf")
                    for ch in range(HC):
                        nc.tensor.matmul(
                            pf_t,
                            wT_sb[:, ch, :],
                            hid_sb[:, s_i, ch, :],
                            start=(ch == 0),
                            stop=(ch == HC - 1),
                        )
                    # exp(feats/WSCALE + b) into M rows 0:52 (cols strided by BL)
                    mview = mchunks[c][0:K, :].rearrange(
                        "p (t b) -> p t b", b=BL
                    )[:, :, b : b + 1]
                    nc.scalar.activation(
                        mview, pf_t, mybir.ActivationFunctionType.Exp,
                        bias=bias_sb, scale=1.0 / WSCALE,
                    )
                    s_i += 1

            # ---- phase B: the 1024-step recursion ----
            for t in range(T):
                c, ti = divmod(t, 128)
                p_t = prpsum.tile([NO, BL], F32, tag="pr")
                nc.tensor.matmul(p_t, trAug_sb, s_cur, start=True, stop=True)
                s_next = spool.tile([NS, BL], F32, tag="state")
                nc.vector.tensor_mul(
                    s_next,
                    mchunks[c][:, ti * BL : (ti + 1) * BL],
                    p_t[0:NS, :],
                )
                if (t + 1) % R == 0:
                    e = (t + 1) // R - 1
                    srec = scap_sb[0:1, e * BL : (e + 1) * BL]
                    nc.vector.reciprocal(srec, p_t[NO - 1 : NO, :])
                    bc_t = pbp.tile([K, BL], F32, tag="pb")
                    nc.tensor.matmul(bc_t, ones_r_sb, srec, start=True, stop=True)
                    nc.vector.tensor_mul(s_next[0:K, :], s_next[0:K, :], bc_t)
                s_cur = s_next

            # ---- outputs ----
            nc.sync.dma_start(sfinal[:, :], s_cur)
            nc.sync.dma_start(scap_d[:, :], scap_sb)

    nc.compile()
    return nc


def kernel(hidden, W, b, log_transitions, tags, lengths):
    hidden = np.asarray(hidden, dtype=np.float32)
    W = np.asarray(W, dtype=np.float32)
    b = np.asarray(b, dtype=np.float32)
    trans = np.asarray(log_transitions, dtype=np.float32)
    tags = np.asarray(tags, dtype=np.int32)
    lengths = np.asarray(lengths, dtype=np.int32)

    C = np.float64(np.exp(LOGC))
    expTr = np.exp(trans.astype(np.float64))
    trAug = np.zeros((NS, NO), dtype=np.float64)
    trAug[:K, :K] = expTr.T / C
    trAug[:K, K] = expTr[END, :] / C          # Z capture column
    trAug[K, K + 1] = 1.0                     # A' = A + Z
    trAug[K + 1, K + 1] = 1.0
    trAug[:K, NO - 1] = 1.0 / C               # Shat column (partition 64)
    trAug = trAug.astype(np.float32)

    cpack = np.zeros((NS, NO + 1), dtype=np.float32)
    cpack[:, 0:NO] = trAug
    cpack[0:K, NO] = b

    # ---- length-ranked round-robin assignment + per-slot chunk budgets ----
    order = np.argsort(-lengths.astype(np.int64), kind="stable")
    Lsort = lengths.astype(np.int64)[order]
    nb = tuple(int(-(-Lsort[bslot * NCORE] // 128)) for bslot in range(BL))
    nslot = sum(nb)
    bsel = np.concatenate([np.full(nb[bslot], bslot) for bslot in range(BL)])
    csel = np.concatenate([np.arange(nb[bslot]) for bslot in range(BL)])

    v = (lengths.astype(np.int64) - 1)        # capture step per sequence
    pos = np.arange(T)[None, :]
    maskT = pos < lengths[:, None]
    is_last = pos == (lengths[:, None] - 1)
    emask = (maskT & ~is_last)

    # ---- fp8 packed, transposed hidden ----
    h8 = hidden.astype(ml_dtypes.float8_e4m3)
    hview = h8.reshape(B, NCHUNK, 128, HC, 128)   # [g, c, x, ch, p]
    wT8 = np.ascontiguousarray(
        (W * WSCALE).astype(ml_dtypes.float8_e4m3)
        .T.reshape(HC, 128, K).transpose(1, 0, 2)
    ).reshape(128, HC * K)

    in_maps = []
    gidx_all = []
    for core in range(NCORE):
        gidx = order[np.arange(BL) * NCORE + core]
        gidx_all.append(gidx)
        sel = hview[gidx[bsel], csel]             # [nslot, x, ch, p]
        hperm = np.ascontiguousarray(sel.transpose(3, 0, 2, 1)).reshape(
            128, nslot * HC * 128
        )
        v_c = v[gidx]
        tt = np.arange(T)
        delta = (tt[:, None] == v_c[None, :]).astype(np.float32)   # [T, BL]
        in_maps.append({
            "hpack": np.concatenate([hperm, wT8], axis=1),
            "cpack": cpack,
            "mdelta": np.ascontiguousarray(delta.reshape(NCHUNK, 128 * BL)),
        })

    if nb not in _NC_CACHE:
        _NC_CACHE[nb] = build_bass(nb)
    nc = _NC_CACHE[nb]

    res = run_bass_kernel_spmd(nc, in_maps, core_ids=list(range(NCORE)))
    outs = res.results

    # ---- host gold score (exact f32): transitions + emissions ----
    tags_ext = np.concatenate(
        [np.full((B, 1), ROOT, tags.dtype), tags], axis=1
    )
    tr_score = (trans[tags, tags_ext[:, :-1]].astype(np.float64) * maskT).sum(axis=1)
    emit_score = np.zeros(B, dtype=np.float64)
    for core in range(NCORE):
        bs = slice(core * BL, (core + 1) * BL)
        Wg = W[tags[bs]]                                     # [BL, T, H]
        ef = np.einsum("bth,bth->bt", hidden[bs], Wg) + b[tags[bs]]
        emit_score[bs] = (ef.astype(np.float64) * emask[bs]).sum(axis=1)

    # ---- assemble nll ----
    nll = np.zeros(B, dtype=np.float64)
    ev_steps = R * np.arange(1, NEV + 1) - 1                 # [NEV]
    for core in range(NCORE):
        gidx = gidx_all[core]
        v_c = v[gidx]
        sfin = outs[core]["sfinal"].astype(np.float64)
        scap = outs[core]["scap"].reshape(NEV, BL).astype(np.float64)
        AZ = sfin[K] + sfin[K + 1]
        prefix_mask = ev_steps[:, None] < v_c[None, :]
        logS_prefix = (-np.log(scap) * prefix_mask).sum(axis=0)
        log_z = np.log(AZ) + (v_c + 1) * LOGC + logS_prefix
        nll[gidx] = log_z - tr_score[gidx] - emit_score[gidx]

    return nll.astype(np.float32)


# revision 5
# speedup vs baseline: 12.4234x; 1.6617x over previous
"""ChainCRF NLL kernel for Trainium2 (8 NeuronCores, data parallel over B).

Transfer-optimized design (the axon tunnel at ~75 MB/s dominates the span):
  - hidden ships as fp8e4m3, host-pre-transposed to [H-chunk, t] layout and
    packed with the (x16-scaled) fp8 W into one DRAM tensor per core.
  - sequences are assigned to cores round-robin by descending-length rank, so
    all cores share one static per-slot chunk budget nb[b] =
    ceil(max-length-in-rank-group-b / 128); only those chunks ship. Columns
    of the M buffer beyond a sequence's budget stay at 1.0 — the recursion
    there decays geometrically and the periodic rescale renormalizes it, so
    the Z/A capture rows are unaffected.
  - gold score (transitions + emissions) is computed exactly on host in f32.
  - device computes feats via fp8 matmul, exp(feats/16 + b) into per-chunk
    M buffers, then runs the exp-domain linear recursion
        Ehat_{t+1} = expFeat_t * (TrAug @ Ehat_t)
    with TrAug carrying the exp(trans)/C block, a Z capture column (selected
    by the host-supplied delta row at t == len[b]-1), an A accumulator
    (A' = A + Z), and a 1/C ones column producing Shat for periodic rescale.
  - host: nll = [log(A+Z) + (v+1)*logC + sum of event logS before v] - gold.

The NEFF is specialized on the budget tuple nb (derived from lengths) and
cached per-process; a different length profile just triggers a recompile.
"""

import os

import numpy as np
import ml_dtypes

import jax

# Persistent XLA compilation cache: run_bass_kernel_spmd rebuilds its jit
# wrapper every call, so without this each call pays a ~0.4 s recompile.
try:
    jax.config.update(
        "jax_compilation_cache_dir", os.path.expanduser("~/.jax_comp_cache")
    )
    jax.config.update("jax_persistent_cache_min_compile_time_secs", 0.0)
    jax.config.update("jax_persistent_cache_min_entry_size_bytes", 0)
except Exception:
    pass

import concourse.bass as bass
import concourse.bacc as bacc
import concourse.tile as tile
from concourse import mybir
from concourse.bass_utils import run_bass_kernel_spmd

B, T, H, K = 128, 1024, 512, 52
ROOT, END = 0, 1
NCORE = 8
BL = B // NCORE          # 16 sequences per core
NS = K + 2               # state rows: 52 Ehat + Z + A
NO = 65                  # out rows: 52 U + Z + A + pad, Shat at partition 64
R = 32                   # rescale period
NEV = T // R             # 32 events
LOGC = 4.9               # constant per-step rescale (exp-domain drift removal)
WSCALE = 16.0            # fp8 range scaling for W; undone by activation scale

NCHUNK = T // 128        # 8 time chunks of 128 steps
HC = H // 128            # 4 H-chunks

F32 = mybir.dt.float32
FP8 = mybir.dt.float8e4

_NC_CACHE = {}


def build_bass(nb):
    nslot = sum(nb)
    hidcol = nslot * HC * 128
    packcol = hidcol + HC * K

    nc = bacc.Bacc(None)
    hpack = nc.dram_tensor("hpack", [128, packcol], FP8, kind="ExternalInput")
    cpack = nc.dram_tensor("cpack", [NS, NO + 1], F32, kind="ExternalInput")
    mdelta = nc.dram_tensor("mdelta", [NCHUNK, 128 * BL], F32, kind="ExternalInput")

    sfinal = nc.dram_tensor("sfinal", [NS, BL], F32, kind="ExternalOutput")
    scap_d = nc.dram_tensor("scap", [1, NEV * BL], F32, kind="ExternalOutput")

    with tile.TileContext(nc) as tc:
        with (
            tc.tile_pool(name="consts", bufs=1) as consts,
            tc.tile_pool(name="mbuf", bufs=1) as mbuf,
            tc.tile_pool(name="state", bufs=3) as spool,
            tc.tile_pool(name="pf", bufs=4, space="PSUM") as pfp,
            tc.tile_pool(name="pr", bufs=2, space="PSUM") as prpsum,
            tc.tile_pool(name="pb", bufs=1, space="PSUM") as pbp,
        ):
            # ---- constants / inputs resident in SBUF ----
            hid_sb = consts.tile([128, nslot, HC, 128], FP8, tag="hid")
            nc.sync.dma_start(
                hid_sb,
                hpack[:, 0:hidcol].rearrange("p (s h x) -> p s h x", s=nslot, h=HC),
            )
            wT_sb = consts.tile([128, HC, K], FP8, tag="wT")
            nc.sync.dma_start(
                wT_sb, hpack[:, hidcol:packcol].rearrange("p (h k) -> p h k", h=HC)
            )
            trAug_sb = consts.tile([NS, NO], F32, tag="trAug")
            nc.sync.dma_start(trAug_sb, cpack[:, 0:NO])
            bias_sb = consts.tile([K, 1], F32, tag="bvec")
            nc.sync.dma_start(bias_sb, cpack[0:K, NO : NO + 1])
            ones_r_sb = consts.tile([1, K], F32, tag="ones_r")
            nc.gpsimd.memset(ones_r_sb, 1.0)
            scap_sb = consts.tile([1, NEV * BL], F32, tag="scap")

            mchunks = []
            for c in range(NCHUNK):
                mc = mbuf.tile([NS, 128 * BL], F32, tag=f"m{c}")
                nc.gpsimd.memset(mc, 1.0)
                nc.sync.dma_start(mc[K : K + 1, :], mdelta[c : c + 1, :])
                mchunks.append(mc)

            s_cur = spool.tile([NS, BL], F32, tag="state")
            nc.gpsimd.memset(s_cur, 0.0)
            nc.gpsimd.memset(s_cur[ROOT : ROOT + 1, :], 1.0)

            # ---- phase A: feats for all budgeted (slot, chunk) pairs ----
            s_i = 0
            for b in range(BL):
                for c in range(nb[b]):
                    pf_t = pfp.tile([K, 128], F32, tag="pf")
                    for ch in range(HC):
                        nc.tensor.matmul(
                            pf_t,
                            wT_sb[:, ch, :],
                            hid_sb[:, s_i, ch, :],
                            start=(ch == 0),
                            stop=(ch == HC - 1),
                        )
                    # exp(feats/WSCALE + b) into M rows 0:52 (cols strided by BL)
                    mview = mchunks[c][0:K, :].rearrange(
                        "p (t b) -> p t b", b=BL
                    )[:, :, b : b + 1]
                    nc.scalar.activation(
                        mview, pf_t, mybir.ActivationFunctionType.Exp,
                        bias=bias_sb, scale=1.0 / WSCALE,
                    )
                    s_i += 1

            # ---- phase B: the 1024-step recursion ----
            for t in range(T):
                c, ti = divmod(t, 128)
                p_t = prpsum.tile([NO, BL], F32, tag="pr")
                nc.tensor.matmul(p_t, trAug_sb, s_cur, start=True, stop=True)
                s_next = spool.tile([NS, BL], F32, tag="state")
                nc.vector.tensor_mul(
                    s_next,
                    mchunks[c][:, ti * BL : (ti + 1) * BL],
                    p_t[0:NS, :],
                )
                if (t + 1) % R == 0:
                    e = (t + 1) // R - 1
                    srec = scap_sb[0:1, e * BL : (e + 1) * BL]
                    nc.vector.reciprocal(srec, p_t[NO - 1 : NO, :])
                    bc_t = pbp.tile([K, BL], F32, tag="pb")
                    nc.tensor.matmul(bc_t, ones_r_sb, srec, start=True, stop=True)
                    nc.vector.tensor_mul(s_next[0:K, :], s_next[0:K, :], bc_t)
                s_cur = s_next

            # ---- outputs ----
            nc.sync.dma_start(sfinal[:, :], s_cur)
            nc.sync.dma_start(scap_d[:, :], scap_sb)

    nc.compile()
    return nc


def kernel(hidden, W, b, log_transitions, tags, lengths):
    hidden = np.asarray(hidden, dtype=np.float32)
    W = np.asarray(W, dtype=np.float32)
    b = np.asarray(b, dtype=np.float32)
    trans = np.asarray(log_transitions, dtype=np.float32)
    tags = np.asarray(tags, dtype=np.int32)
    lengths = np.asarray(lengths, dtype=np.int32)

    C = np.float64(np.exp(LOGC))
    expTr = np.exp(trans.astype(np.float64))
    trAug = np.zeros((NS, NO), dtype=np.float64)
    trAug[:K, :K] = expTr.T / C
    trAug[:K, K] = expTr[END, :] / C          # Z capture column
    trAug[K, K + 1] = 1.0                     # A' = A + Z
    trAug[K + 1, K + 1] = 1.0
    trAug[:K, NO - 1] = 1.0 / C               # Shat column (partition 64)
    trAug = trAug.astype(np.float32)

    cpack = np.zeros((NS, NO + 1), dtype=np.float32)
    cpack[:, 0:NO] = trAug
    cpack[0:K, NO] = b

    # ---- length-ranked round-robin assignment + per-slot chunk budgets ----
    order = np.argsort(-lengths.astype(np.int64), kind="stable")
    Lsort = lengths.astype(np.int64)[order]
    nb = tuple(int(-(-Lsort[bslot * NCORE] // 128)) for bslot in range(BL))
    nslot = sum(nb)
    bsel = np.concatenate([np.full(nb[bslot], bslot) for bslot in range(BL)])
    csel = np.concatenate([np.arange(nb[bslot]) for bslot in range(BL)])

    v = (lengths.astype(np.int64) - 1)        # capture step per sequence
    pos = np.arange(T)[None, :]
    maskT = pos < lengths[:, None]
    is_last = pos == (lengths[:, None] - 1)
    emask = (maskT & ~is_last)

    # ---- fp8 packed, transposed hidden ----
    h8 = hidden.astype(ml_dtypes.float8_e4m3)
    hview = h8.reshape(B, NCHUNK, 128, HC, 128)   # [g, c, x, ch, p]
    wT8 = np.ascontiguousarray(
        (W * WSCALE).astype(ml_dtypes.float8_e4m3)
        .T.reshape(HC, 128, K).transpose(1, 0, 2)
    ).reshape(128, HC * K)

    in_maps = []
    gidx_all = []
    for core in range(NCORE):
        gidx = order[np.arange(BL) * NCORE + core]
        gidx_all.append(gidx)
        sel = hview[gidx[bsel], csel]             # [nslot, x, ch, p]
        hperm = np.ascontiguousarray(sel.transpose(3, 0, 2, 1)).reshape(
            128, nslot * HC * 128
        )
        v_c = v[gidx]
        tt = np.arange(T)
        delta = (tt[:, None] == v_c[None, :]).astype(np.float32)   # [T, BL]
        in_maps.append({
            "hpack": np.concatenate([hperm, wT8], axis=1),
            "cpack": cpack,
            "mdelta": np.ascontiguousarray(delta.reshape(NCHUNK, 128 * BL)),
        })

    if nb not in _NC_CACHE:
        _NC_CACHE[nb] = build_bass(nb)
    nc = _NC_CACHE[nb]

    res = run_bass_kernel_spmd(nc, in_maps, core_ids=list(range(NCORE)))
    outs = res.results

    # ---- host gold score (exact f32): transitions + emissions ----
    tags_ext = np.concatenate(
        [np.full((B, 1), ROOT, tags.dtype), tags], axis=1
    )
    tr_score = (trans[tags, tags_ext[:, :-1]].astype(np.float64) * maskT).sum(axis=1)
    emit_score = np.zeros(B, dtype=np.float64)
    for core in range(NCORE):
        bs = slice(core * BL, (core + 1) * BL)
        Wg = W[tags[bs]]                                     # [BL, T, H]
        ef = np.einsum("bth,bth->bt", hidden[bs], Wg) + b[tags[bs]]
        emit_score[bs] = (ef.astype(np.float64) * emask[bs]).sum(axis=1)

    # ---- assemble nll ----
    nll = np.zeros(B, dtype=np.float64)
    ev_steps = R * np.arange(1, NEV + 1) - 1                 # [NEV]
    for core in range(NCORE):
        gidx = gidx_all[core]
        v_c = v[gidx]
        sfin = outs[core]["sfinal"].astype(np.float64)
        scap = outs[core]["scap"].reshape(NEV, BL).astype(np.float64)
        AZ = sfin[K] + sfin[K + 1]
        prefix_mask = ev_steps[:, None] < v_c[None, :]
        logS_prefix = (-np.log(scap) * prefix_mask).sum(axis=0)
        log_z = np.log(AZ) + (v_c + 1) * LOGC + logS_prefix
        nll[gidx] = log_z - tr_score[gidx] - emit_score[gidx]

    return nll.astype(np.float32)


# revision 11
# speedup vs baseline: 12.7176x; 1.0237x over previous
"""ChainCRF NLL kernel for Trainium2 (8 NeuronCores, data parallel over B).

Transfer-optimized design (the axon tunnel at ~75 MB/s dominates the span):
  - hidden ships as fp8e4m3, host-pre-transposed to [H-chunk, t] layout and
    packed with the (x16-scaled) fp8 W into one DRAM tensor per core.
  - sequences are assigned to cores round-robin by descending-length rank, so
    all cores share one static per-slot chunk budget nb[b] =
    ceil(max-length-in-rank-group-b / 128); only those chunks ship. Columns
    of the M buffer beyond a sequence's budget stay at 1.0 — the recursion
    there decays geometrically and the periodic rescale renormalizes it, so
    the Z/A capture rows are unaffected.
  - gold score (transitions + emissions) is computed exactly on host in f32.
  - device computes feats via fp8 matmul, exp(feats/16 + b) into per-chunk
    M buffers, then runs the exp-domain linear recursion
        Ehat_{t+1} = expFeat_t * (TrAug @ Ehat_t)
    with TrAug carrying the exp(trans)/C block, a Z capture column (selected
    by the host-supplied delta row at t == len[b]-1), an A accumulator
    (A' = A + Z), and a 1/C ones column producing Shat for periodic rescale.
  - host: nll = [log(A+Z) + (v+1)*logC + sum of event logS before v] - gold.

The NEFF is specialized on the budget tuple nb (derived from lengths) and
cached per-process; a different length profile just triggers a recompile.
"""

import os

import numpy as np
import ml_dtypes

import jax

# Persistent XLA compilation cache: run_bass_kernel_spmd rebuilds its jit
# wrapper every call, so without this each call pays a ~0.4 s recompile.
try:
    jax.config.update(
        "jax_compilation_cache_dir", os.path.expanduser("~/.jax_comp_cache")
    )
    jax.config.update("jax_persistent_cache_min_compile_time_secs", 0.0)
    jax.config.update("jax_persistent_cache_min_entry_size_bytes", 0)
except Exception:
    pass

import concourse.bass as bass
import concourse.bacc as bacc
import concourse.tile as tile
from concourse import mybir
from concourse.bass_utils import run_bass_kernel_spmd

B, T, H, K = 128, 1024, 512, 52
ROOT, END = 0, 1
NCORE = 8
BL = B // NCORE          # 16 sequences per core
NS = K + 2               # state rows: 52 Ehat + Z + A
NO = 65                  # out rows: 52 U + Z + A + pad, Shat at partition 64
R = 32                   # rescale period
NEV = T // R             # 32 events
LOGC = 4.9               # constant per-step rescale (exp-domain drift removal)
WSCALE = 16.0            # fp8 range scaling for W; undone by activation scale

NCHUNK = T // 128        # 8 time chunks of 128 steps
HC = H // 128            # 4 H-chunks

F32 = mybir.dt.float32
FP8 = mybir.dt.float8e4

_NC_CACHE = {}


def build_bass(nb):
    # nb[b] = per-slot budget in 64-step units
    nslot = sum(nb)
    hidcol = nslot * HC * 64
    packcol = hidcol + HC * K

    nc = bacc.Bacc(None)
    hpack = nc.dram_tensor("hpack", [128, packcol], FP8, kind="ExternalInput")
    cpack = nc.dram_tensor("cpack", [NS, NO + 1], F32, kind="ExternalInput")
    mdelta = nc.dram_tensor("mdelta", [NCHUNK, 128 * BL], F32, kind="ExternalInput")

    sfinal = nc.dram_tensor("sfinal", [NS, BL], F32, kind="ExternalOutput")
    scap_d = nc.dram_tensor("scap", [1, NEV * BL], F32, kind="ExternalOutput")

    with tile.TileContext(nc) as tc:
        with (
            tc.tile_pool(name="consts", bufs=1) as consts,
            tc.tile_pool(name="mbuf", bufs=1) as mbuf,
            tc.tile_pool(name="state", bufs=3) as spool,
            tc.tile_pool(name="pf", bufs=4, space="PSUM") as pfp,
            tc.tile_pool(name="pr", bufs=2, space="PSUM") as prpsum,
            tc.tile_pool(name="pb", bufs=1, space="PSUM") as pbp,
        ):
            # ---- constants / inputs resident in SBUF ----
            hid_sb = consts.tile([128, nslot, HC, 64], FP8, tag="hid")
            nc.sync.dma_start(
                hid_sb,
                hpack[:, 0:hidcol].rearrange("p (s h x) -> p s h x", s=nslot, h=HC),
            )
            wT_sb = consts.tile([128, HC, K], FP8, tag="wT")
            nc.sync.dma_start(
                wT_sb, hpack[:, hidcol:packcol].rearrange("p (h k) -> p h k", h=HC)
            )
            trAug_sb = consts.tile([NS, NO], F32, tag="trAug")
            nc.sync.dma_start(trAug_sb, cpack[:, 0:NO])
            bias_sb = consts.tile([K, 1], F32, tag="bvec")
            nc.sync.dma_start(bias_sb, cpack[0:K, NO : NO + 1])
            ones_r_sb = consts.tile([1, K], F32, tag="ones_r")
            nc.gpsimd.memset(ones_r_sb, 1.0)
            scap_sb = consts.tile([1, NEV * BL], F32, tag="scap")

            mchunks = []
            for c in range(NCHUNK):
                mc = mbuf.tile([NS, 128 * BL], F32, tag=f"m{c}")
                nc.gpsimd.memset(mc, 1.0)
                nc.sync.dma_start(mc[K : K + 1, :], mdelta[c : c + 1, :])
                mchunks.append(mc)

            s_cur = spool.tile([NS, BL], F32, tag="state")
            nc.gpsimd.memset(s_cur, 0.0)
            nc.gpsimd.memset(s_cur[ROOT : ROOT + 1, :], 1.0)

            # ---- phase A: feats for all budgeted 64-step (slot, q) pieces ----
            s_i = 0
            for b in range(BL):
                for q in range(nb[b]):
                    c, half = divmod(q, 2)
                    pf_t = pfp.tile([K, 64], F32, tag="pf")
                    for ch in range(HC):
                        nc.tensor.matmul(
                            pf_t,
                            wT_sb[:, ch, :],
                            hid_sb[:, s_i, ch, :],
                            start=(ch == 0),
                            stop=(ch == HC - 1),
                        )
                    # exp(feats/WSCALE + b) into M rows 0:52 (cols strided by BL)
                    mview = mchunks[c][0:K, :].rearrange(
                        "p (t b) -> p t b", b=BL
                    )[:, half * 64 : (half + 1) * 64, b : b + 1]
                    nc.scalar.activation(
                        mview, pf_t, mybir.ActivationFunctionType.Exp,
                        bias=bias_sb, scale=1.0 / WSCALE,
                    )
                    s_i += 1

            # ---- phase B: the 1024-step recursion ----
            for t in range(T):
                c, ti = divmod(t, 128)
                p_t = prpsum.tile([NO, BL], F32, tag="pr")
                nc.tensor.matmul(p_t, trAug_sb, s_cur, start=True, stop=True)
                s_next = spool.tile([NS, BL], F32, tag="state")
                nc.vector.tensor_mul(
                    s_next,
                    mchunks[c][:, ti * BL : (ti + 1) * BL],
                    p_t[0:NS, :],
                )
                if (t + 1) % R == 0:
                    e = (t + 1) // R - 1
                    srec = scap_sb[0:1, e * BL : (e + 1) * BL]
                    nc.vector.reciprocal(srec, p_t[NO - 1 : NO, :])
                    bc_t = pbp.tile([K, BL], F32, tag="pb")
                    nc.tensor.matmul(bc_t, ones_r_sb, srec, start=True, stop=True)
                    nc.vector.tensor_mul(s_next[0:K, :], s_next[0:K, :], bc_t)
                s_cur = s_next

            # ---- outputs ----
            nc.sync.dma_start(sfinal[:, :], s_cur)
            nc.sync.dma_start(scap_d[:, :], scap_sb)

    nc.compile()
    return nc


def kernel(hidden, W, b, log_transitions, tags, lengths):
    hidden = np.asarray(hidden, dtype=np.float32)
    W = np.asarray(W, dtype=np.float32)
    b = np.asarray(b, dtype=np.float32)
    trans = np.asarray(log_transitions, dtype=np.float32)
    tags = np.asarray(tags, dtype=np.int32)
    lengths = np.asarray(lengths, dtype=np.int32)

    C = np.float64(np.exp(LOGC))
    expTr = np.exp(trans.astype(np.float64))
    trAug = np.zeros((NS, NO), dtype=np.float64)
    trAug[:K, :K] = expTr.T / C
    trAug[:K, K] = expTr[END, :] / C          # Z capture column
    trAug[K, K + 1] = 1.0                     # A' = A + Z
    trAug[K + 1, K + 1] = 1.0
    trAug[:K, NO - 1] = 1.0 / C               # Shat column (partition 64)
    trAug = trAug.astype(np.float32)

    cpack = np.zeros((NS, NO + 1), dtype=np.float32)
    cpack[:, 0:NO] = trAug
    cpack[0:K, NO] = b

    # ---- length-ranked round-robin assignment + per-slot 64-step budgets ----
    order = np.argsort(-lengths.astype(np.int64), kind="stable")
    Lsort = lengths.astype(np.int64)[order]
    nb = tuple(int(-(-Lsort[bslot * NCORE] // 64)) for bslot in range(BL))
    nslot = sum(nb)
    bsel = np.concatenate([np.full(nb[bslot], bslot) for bslot in range(BL)])
    qsel = np.concatenate([np.arange(nb[bslot]) for bslot in range(BL)])

    v = (lengths.astype(np.int64) - 1)        # capture step per sequence
    pos = np.arange(T)[None, :]
    maskT = pos < lengths[:, None]
    is_last = pos == (lengths[:, None] - 1)
    emask = (maskT & ~is_last)

    # ---- fp8 packed, transposed hidden ----
    h8 = hidden.astype(ml_dtypes.float8_e4m3)
    hview = h8.reshape(B, 2 * NCHUNK, 64, HC, 128)   # [g, q, x, ch, p]
    wT8 = np.ascontiguousarray(
        (W * WSCALE).astype(ml_dtypes.float8_e4m3)
        .T.reshape(HC, 128, K).transpose(1, 0, 2)
    ).reshape(128, HC * K)

    in_maps = []
    gidx_all = []
    for core in range(NCORE):
        gidx = order[np.arange(BL) * NCORE + core]
        gidx_all.append(gidx)
        sel = hview[gidx[bsel], qsel]             # [nslot, x, ch, p]
        hperm = np.ascontiguousarray(sel.transpose(3, 0, 2, 1)).reshape(
            128, nslot * HC * 64
        )
        v_c = v[gidx]
        tt = np.arange(T)
        delta = (tt[:, None] == v_c[None, :]).astype(np.float32)   # [T, BL]
        in_maps.append({
            "hpack": np.concatenate([hperm, wT8], axis=1),
            "cpack": cpack,
            "mdelta": np.ascontiguousarray(delta.reshape(NCHUNK, 128 * BL)),
        })

    if nb not in _NC_CACHE:
        _NC_CACHE[nb] = build_bass(nb)
    nc = _NC_CACHE[nb]

    res = run_bass_kernel_spmd(nc, in_maps, core_ids=list(range(NCORE)))
    outs = res.results

    # ---- host gold score (exact f32): transitions + emissions ----
    tags_ext = np.concatenate(
        [np.full((B, 1), ROOT, tags.dtype), tags], axis=1
    )
    tr_score = (trans[tags, tags_ext[:, :-1]].astype(np.float64) * maskT).sum(axis=1)
    emit_score = np.zeros(B, dtype=np.float64)
    for core in range(NCORE):
        bs = slice(core * BL, (core + 1) * BL)
        Wg = W[tags[bs]]                                     # [BL, T, H]
        ef = np.einsum("bth,bth->bt", hidden[bs], Wg) + b[tags[bs]]
        emit_score[bs] = (ef.astype(np.float64) * emask[bs]).sum(axis=1)

    # ---- assemble nll ----
    nll = np.zeros(B, dtype=np.float64)
    ev_steps = R * np.arange(1, NEV + 1) - 1                 # [NEV]
    for core in range(NCORE):
        gidx = gidx_all[core]
        v_c = v[gidx]
        sfin = outs[core]["sfinal"].astype(np.float64)
        scap = outs[core]["scap"].reshape(NEV, BL).astype(np.float64)
        AZ = sfin[K] + sfin[K + 1]
        prefix_mask = ev_steps[:, None] < v_c[None, :]
        logS_prefix = (-np.log(scap) * prefix_mask).sum(axis=0)
        log_z = np.log(AZ) + (v_c + 1) * LOGC + logS_prefix
        nll[gidx] = log_z - tr_score[gidx] - emit_score[gidx]

    return nll.astype(np.float32)


# revision 18
# speedup vs baseline: 13.4445x; 1.0572x over previous
"""ChainCRF NLL kernel for Trainium2 (8 NeuronCores, data parallel over B).

Transfer-optimized design (the axon tunnel at ~75 MB/s dominates the span):
  - hidden ships as fp8e4m3, host-pre-transposed to [H-chunk, t] layout and
    packed with the (x16-scaled) fp8 W into one DRAM tensor per core.
  - sequences are assigned to cores round-robin by descending-length rank, so
    all cores share one static per-slot chunk budget nb[b] =
    ceil(max-length-in-rank-group-b / 128); only those chunks ship. Columns
    of the M buffer beyond a sequence's budget stay at 1.0 — the recursion
    there decays geometrically and the periodic rescale renormalizes it, so
    the Z/A capture rows are unaffected.
  - gold score (transitions + emissions) is computed exactly on host in f32.
  - device computes feats via fp8 matmul, exp(feats/16 + b) into per-chunk
    M buffers, then runs the exp-domain linear recursion
        Ehat_{t+1} = expFeat_t * (TrAug @ Ehat_t)
    with TrAug carrying the exp(trans)/C block, a Z capture column (selected
    by the host-supplied delta row at t == len[b]-1), an A accumulator
    (A' = A + Z), and a 1/C ones column producing Shat for periodic rescale.
  - host: nll = [log(A+Z) + (v+1)*logC + sum of event logS before v] - gold.

The NEFF is specialized on the budget tuple nb (derived from lengths) and
cached per-process; a different length profile just triggers a recompile.
"""

import os

import numpy as np
import ml_dtypes

import jax

# Persistent XLA compilation cache: run_bass_kernel_spmd rebuilds its jit
# wrapper every call, so without this each call pays a ~0.4 s recompile.
try:
    jax.config.update(
        "jax_compilation_cache_dir", os.path.expanduser("~/.jax_comp_cache")
    )
    jax.config.update("jax_persistent_cache_min_compile_time_secs", 0.0)
    jax.config.update("jax_persistent_cache_min_entry_size_bytes", 0)
except Exception:
    pass

import concourse.bass as bass
import concourse.bacc as bacc
import concourse.tile as tile
from concourse import mybir
from concourse.bass_utils import run_bass_kernel_spmd

B, T, H, K = 128, 1024, 512, 52
ROOT, END = 0, 1
NCORE = 8
BL = B // NCORE          # 16 sequences per core
NS = K + 2               # state rows: 52 Ehat + Z + A
NO = 65                  # out rows: 52 U + Z + A + pad, Shat at partition 64
R = 32                   # rescale period
NEV = T // R             # 32 events
LOGC = 4.9               # constant per-step rescale (exp-domain drift removal)
WSCALE = 16.0            # fp8 range scaling for W; undone by activation scale

NCHUNK = T // 128        # 8 time chunks of 128 steps
HC = H // 128            # 4 H-chunks

F32 = mybir.dt.float32
FP8 = mybir.dt.float8e4

_NC_CACHE = {}


def _pieces(wvals):
    """Chunk-boundary pieces (b, c, w_p, off) of the exact-length packing."""
    pieces = []
    off = 0
    for b, w_b in enumerate(wvals):
        for c in range(-(-w_b // 128)):
            w_p = min(128, w_b - c * 128)
            pieces.append((b, c, w_p, off))
            off += HC * w_p
    return pieces, off


def build_bass(wvals):
    # wvals[b] = per-slot packed timestep count (multiple of 4)
    pieces, hidcol = _pieces(wvals)
    packcol = hidcol + HC * K

    nc = bacc.Bacc(None)
    hpack = nc.dram_tensor("hpack", [128, packcol], FP8, kind="ExternalInput")
    cpack = nc.dram_tensor("cpack", [NS, NO + 1], F32, kind="ExternalInput")
    mdelta = nc.dram_tensor("mdelta", [NCHUNK, 128 * BL], F32, kind="ExternalInput")

    sfinal = nc.dram_tensor("sfinal", [NS, BL], F32, kind="ExternalOutput")
    scap_d = nc.dram_tensor("scap", [1, NEV * BL], F32, kind="ExternalOutput")

    with tile.TileContext(nc) as tc:
        with (
            tc.tile_pool(name="consts", bufs=1) as consts,
            tc.tile_pool(name="mbuf", bufs=1) as mbuf,
            tc.tile_pool(name="state", bufs=3) as spool,
            tc.tile_pool(name="pf", bufs=4, space="PSUM") as pfp,
            tc.tile_pool(name="pr", bufs=2, space="PSUM") as prpsum,
            tc.tile_pool(name="pb", bufs=1, space="PSUM") as pbp,
        ):
            # ---- constants / inputs resident in SBUF ----
            hid_sb = consts.tile([128, hidcol], FP8, tag="hid")
            nc.sync.dma_start(hid_sb, hpack[:, 0:hidcol])
            wT_sb = consts.tile([128, HC, K], FP8, tag="wT")
            nc.sync.dma_start(
                wT_sb, hpack[:, hidcol:packcol].rearrange("p (h k) -> p h k", h=HC)
            )
            trAug_sb = consts.tile([NS, NO], F32, tag="trAug")
            nc.sync.dma_start(trAug_sb, cpack[:, 0:NO])
            bias_sb = consts.tile([K, 1], F32, tag="bvec")
            nc.sync.dma_start(bias_sb, cpack[0:K, NO : NO + 1])
            ones_r_sb = consts.tile([1, K], F32, tag="ones_r")
            nc.gpsimd.memset(ones_r_sb, 1.0)
            scap_sb = consts.tile([1, NEV * BL], F32, tag="scap")

            mchunks = []
            for c in range(NCHUNK):
                mc = mbuf.tile([NS, 128 * BL], F32, tag=f"m{c}")
                nc.gpsimd.memset(mc, 1.0)
                nc.sync.dma_start(mc[K : K + 1, :], mdelta[c : c + 1, :])
                mchunks.append(mc)

            s_cur = spool.tile([NS, BL], F32, tag="state")
            nc.gpsimd.memset(s_cur, 0.0)
            nc.gpsimd.memset(s_cur[ROOT : ROOT + 1, :], 1.0)

            # ---- phase A: feats for all packed pieces ----
            for b, c, w_p, off in pieces:
                pf_t = pfp.tile([K, 128], F32, tag="pf")
                for ch in range(HC):
                    nc.tensor.matmul(
                        pf_t[:, 0:w_p],
                        wT_sb[:, ch, :],
                        hid_sb[:, off + ch * w_p : off + (ch + 1) * w_p],
                        start=(ch == 0),
                        stop=(ch == HC - 1),
                    )
                # exp(feats/WSCALE + b) into M rows 0:52 (cols strided by BL)
                mview = mchunks[c][0:K, :].rearrange(
                    "p (t b) -> p t b", b=BL
                )[:, 0:w_p, b : b + 1]
                nc.scalar.activation(
                    mview, pf_t[:, 0:w_p], mybir.ActivationFunctionType.Exp,
                    bias=bias_sb, scale=1.0 / WSCALE,
                )

            # ---- phase B: the 1024-step recursion ----
            for t in range(T):
                c, ti = divmod(t, 128)
                p_t = prpsum.tile([NO, BL], F32, tag="pr")
                nc.tensor.matmul(p_t, trAug_sb, s_cur, start=True, stop=True)
                s_next = spool.tile([NS, BL], F32, tag="state")
                nc.vector.tensor_mul(
                    s_next,
                    mchunks[c][:, ti * BL : (ti + 1) * BL],
                    p_t[0:NS, :],
                )
                if (t + 1) % R == 0:
                    e = (t + 1) // R - 1
                    srec = scap_sb[0:1, e * BL : (e + 1) * BL]
                    nc.vector.reciprocal(srec, p_t[NO - 1 : NO, :])
                    bc_t = pbp.tile([K, BL], F32, tag="pb")
                    nc.tensor.matmul(bc_t, ones_r_sb, srec, start=True, stop=True)
                    nc.vector.tensor_mul(s_next[0:K, :], s_next[0:K, :], bc_t)
                s_cur = s_next

            # ---- outputs ----
            nc.sync.dma_start(sfinal[:, :], s_cur)
            nc.sync.dma_start(scap_d[:, :], scap_sb)

    nc.compile()
    return nc


def kernel(hidden, W, b, log_transitions, tags, lengths):
    hidden = np.asarray(hidden, dtype=np.float32)
    W = np.asarray(W, dtype=np.float32)
    b = np.asarray(b, dtype=np.float32)
    trans = np.asarray(log_transitions, dtype=np.float32)
    tags = np.asarray(tags, dtype=np.int32)
    lengths = np.asarray(lengths, dtype=np.int32)

    C = np.float64(np.exp(LOGC))
    expTr = np.exp(trans.astype(np.float64))
    trAug = np.zeros((NS, NO), dtype=np.float64)
    trAug[:K, :K] = expTr.T / C
    trAug[:K, K] = expTr[END, :] / C          # Z capture column
    trAug[K, K + 1] = 1.0                     # A' = A + Z
    trAug[K + 1, K + 1] = 1.0
    trAug[:K, NO - 1] = 1.0 / C               # Shat column (partition 64)
    trAug = trAug.astype(np.float32)

    cpack = np.zeros((NS, NO + 1), dtype=np.float32)
    cpack[:, 0:NO] = trAug
    cpack[0:K, NO] = b

    # ---- length-ranked round-robin assignment + exact per-slot widths ----
    order = np.argsort(-lengths.astype(np.int64), kind="stable")
    Lsort = lengths.astype(np.int64)[order]
    wvals = tuple(
        min(T, int(-(-Lsort[bslot * NCORE] // 4)) * 4) for bslot in range(BL)
    )
    pieces, hidcol = _pieces(wvals)

    v = (lengths.astype(np.int64) - 1)        # capture step per sequence
    pos = np.arange(T)[None, :]
    maskT = pos < lengths[:, None]
    is_last = pos == (lengths[:, None] - 1)
    emask = (maskT & ~is_last)

    # ---- fp8 packed, transposed hidden ----
    h8 = hidden.astype(ml_dtypes.float8_e4m3)
    wT8 = np.ascontiguousarray(
        (W * WSCALE).astype(ml_dtypes.float8_e4m3)
        .T.reshape(HC, 128, K).transpose(1, 0, 2)
    ).reshape(128, HC * K)

    in_maps = []
    gidx_all = []
    for core in range(NCORE):
        gidx = order[np.arange(BL) * NCORE + core]
        gidx_all.append(gidx)
        hpack = np.empty((128, hidcol + HC * K), dtype=ml_dtypes.float8_e4m3)
        for bslot, c_i, w_p, off in pieces:
            t0 = c_i * 128
            block = h8[gidx[bslot], t0 : t0 + w_p, :].reshape(w_p, HC, 128)
            hpack[:, off : off + HC * w_p] = (
                block.transpose(2, 1, 0).reshape(128, HC * w_p)
            )
        hpack[:, hidcol:] = wT8
        v_c = v[gidx]
        tt = np.arange(T)
        delta = (tt[:, None] == v_c[None, :]).astype(np.float32)   # [T, BL]
        in_maps.append({
            "hpack": hpack,
            "cpack": cpack,
            "mdelta": np.ascontiguousarray(delta.reshape(NCHUNK, 128 * BL)),
        })

    if wvals not in _NC_CACHE:
        _NC_CACHE[wvals] = build_bass(wvals)
    nc = _NC_CACHE[wvals]

    res = run_bass_kernel_spmd(nc, in_maps, core_ids=list(range(NCORE)))
    outs = res.results

    # ---- host gold score (exact f32): transitions + emissions ----
    tags_ext = np.concatenate(
        [np.full((B, 1), ROOT, tags.dtype), tags], axis=1
    )
    tr_score = (trans[tags, tags_ext[:, :-1]].astype(np.float64) * maskT).sum(axis=1)
    emit_score = np.zeros(B, dtype=np.float64)
    for core in range(NCORE):
        bs = slice(core * BL, (core + 1) * BL)
        Wg = W[tags[bs]]                                     # [BL, T, H]
        ef = np.einsum("bth,bth->bt", hidden[bs], Wg) + b[tags[bs]]
        emit_score[bs] = (ef.astype(np.float64) * emask[bs]).sum(axis=1)

    # ---- assemble nll ----
    nll = np.zeros(B, dtype=np.float64)
    ev_steps = R * np.arange(1, NEV + 1) - 1                 # [NEV]
    for core in range(NCORE):
        gidx = gidx_all[core]
        v_c = v[gidx]
        sfin = outs[core]["sfinal"].astype(np.float64)
        scap = outs[core]["scap"].reshape(NEV, BL).astype(np.float64)
        AZ = sfin[K] + sfin[K + 1]
        prefix_mask = ev_steps[:, None] < v_c[None, :]
        logS_prefix = (-np.log(scap) * prefix_mask).sum(axis=0)
        log_z = np.log(AZ) + (v_c + 1) * LOGC + logS_prefix
        nll[gidx] = log_z - tr_score[gidx] - emit_score[gidx]

    return nll.astype(np.float32)


# revision 23
# speedup vs baseline: 13.8880x; 1.0330x over previous
"""ChainCRF NLL kernel for Trainium2 (8 NeuronCores, data parallel over B).

Transfer-optimized design (the axon tunnel at ~75 MB/s dominates the span):
  - hidden ships as fp8e4m3, host-pre-transposed to [H-chunk, t] layout and
    packed with the (x16-scaled) fp8 W into one DRAM tensor per core.
  - sequences are assigned to cores round-robin by descending-length rank, so
    all cores share one static per-slot chunk budget nb[b] =
    ceil(max-length-in-rank-group-b / 128); only those chunks ship. Columns
    of the M buffer beyond a sequence's budget stay at 1.0 — the recursion
    there decays geometrically and the periodic rescale renormalizes it, so
    the Z/A capture rows are unaffected.
  - gold score (transitions + emissions) is computed exactly on host in f32.
  - device computes feats via fp8 matmul, exp(feats/16 + b) into per-chunk
    M buffers, then runs the exp-domain linear recursion
        Ehat_{t+1} = expFeat_t * (TrAug @ Ehat_t)
    with TrAug carrying the exp(trans)/C block, a Z capture column (selected
    by the host-supplied delta row at t == len[b]-1), an A accumulator
    (A' = A + Z), and a 1/C ones column producing Shat for periodic rescale.
  - host: nll = [log(A+Z) + (v+1)*logC + sum of event logS before v] - gold.

The NEFF is specialized on the budget tuple nb (derived from lengths) and
cached per-process; a different length profile just triggers a recompile.
"""

import os

import numpy as np
import ml_dtypes

import jax

# Persistent XLA compilation cache: run_bass_kernel_spmd rebuilds its jit
# wrapper every call, so without this each call pays a ~0.4 s recompile.
try:
    jax.config.update(
        "jax_compilation_cache_dir", os.path.expanduser("~/.jax_comp_cache")
    )
    jax.config.update("jax_persistent_cache_min_compile_time_secs", 0.0)
    jax.config.update("jax_persistent_cache_min_entry_size_bytes", 0)
except Exception:
    pass

import concourse.bass as bass
import concourse.bacc as bacc
import concourse.tile as tile
from concourse import mybir
from concourse.bass_utils import run_bass_kernel_spmd

B, T, H, K = 128, 1024, 512, 52
ROOT, END = 0, 1
NCORE = 8
BL = B // NCORE          # 16 sequences per core
NS = K + 2               # state rows: 52 Ehat + Z + A
NO = 65                  # out rows: 52 U + Z + A + pad, Shat at partition 64
R = 32                   # rescale period
NEV = T // R             # 32 events
LOGC = 4.9               # constant per-step rescale (exp-domain drift removal)
WSCALE = 16.0            # fp8 range scaling for W; undone by activation scale

NCHUNK = T // 128        # 8 time chunks of 128 steps
HC = H // 128            # 4 H-chunks

F32 = mybir.dt.float32
FP8 = mybir.dt.float8e4

_NC_CACHE = {}


def _pieces(wvals):
    """Chunk-boundary pieces (b, c, w_p, off) of the exact-length packing."""
    pieces = []
    off = 0
    for b, w_b in enumerate(wvals):
        for c in range(-(-w_b // 128)):
            w_p = min(128, w_b - c * 128)
            pieces.append((b, c, w_p, off))
            off += HC * w_p
    return pieces, off


def build_bass(wvals):
    # wvals[b] = per-slot packed timestep count (multiple of 4)
    pieces, hidcol = _pieces(wvals)
    packcol = hidcol + HC * K

    nc = bacc.Bacc(None)
    hpack = nc.dram_tensor("hpack", [128, packcol], FP8, kind="ExternalInput")
    # flat f32 side input: [trAug p-major 54*65 | bias 52 | pad 2 | delta 8*2048]
    FLATN = NS * NO + K + 2 + NCHUNK * 128 * BL
    cpack = nc.dram_tensor("cpack", [1, FLATN], F32, kind="ExternalInput")
    DOFF = NS * NO + K + 2

    sfinal = nc.dram_tensor("sfinal", [NS, BL], F32, kind="ExternalOutput")
    scap_d = nc.dram_tensor("scap", [1, NEV * BL], F32, kind="ExternalOutput")

    with tile.TileContext(nc) as tc:
        with (
            tc.tile_pool(name="consts", bufs=1) as consts,
            tc.tile_pool(name="mbuf", bufs=1) as mbuf,
            tc.tile_pool(name="state", bufs=3) as spool,
            tc.tile_pool(name="pf", bufs=4, space="PSUM") as pfp,
            tc.tile_pool(name="pr", bufs=2, space="PSUM") as prpsum,
            tc.tile_pool(name="pb", bufs=1, space="PSUM") as pbp,
        ):
            # ---- constants / inputs resident in SBUF ----
            hid_sb = consts.tile([128, hidcol], FP8, tag="hid")
            nc.sync.dma_start(hid_sb, hpack[:, 0:hidcol])
            wT_sb = consts.tile([128, HC, K], FP8, tag="wT")
            nc.sync.dma_start(
                wT_sb, hpack[:, hidcol:packcol].rearrange("p (h k) -> p h k", h=HC)
            )
            trAug_sb = consts.tile([NS, NO], F32, tag="trAug")
            nc.sync.dma_start(
                trAug_sb,
                cpack[:, 0 : NS * NO].rearrange("a (p x) -> (a p) x", p=NS),
            )
            bias_sb = consts.tile([K, 1], F32, tag="bvec")
            nc.sync.dma_start(
                bias_sb,
                cpack[:, NS * NO : NS * NO + K].rearrange(
                    "a (p x) -> (a p) x", p=K
                ),
            )
            ones_r_sb = consts.tile([1, K], F32, tag="ones_r")
            nc.gpsimd.memset(ones_r_sb, 1.0)
            scap_sb = consts.tile([1, NEV * BL], F32, tag="scap")

            mchunks = []
            for c in range(NCHUNK):
                mc = mbuf.tile([NS, 128 * BL], F32, tag=f"m{c}")
                nc.gpsimd.memset(mc, 1.0)
                nc.sync.dma_start(
                    mc[K : K + 1, :],
                    cpack[:, DOFF + c * 128 * BL : DOFF + (c + 1) * 128 * BL],
                )
                mchunks.append(mc)

            s_cur = spool.tile([NS, BL], F32, tag="state")
            nc.gpsimd.memset(s_cur, 0.0)
            nc.gpsimd.memset(s_cur[ROOT : ROOT + 1, :], 1.0)

            # ---- phase A: feats for all packed pieces ----
            for b, c, w_p, off in pieces:
                pf_t = pfp.tile([K, 128], F32, tag="pf")
                for ch in range(HC):
                    nc.tensor.matmul(
                        pf_t[:, 0:w_p],
                        wT_sb[:, ch, :],
                        hid_sb[:, off + ch * w_p : off + (ch + 1) * w_p],
                        start=(ch == 0),
                        stop=(ch == HC - 1),
                    )
                # exp(feats/WSCALE + b) into M rows 0:52 (cols strided by BL)
                mview = mchunks[c][0:K, :].rearrange(
                    "p (t b) -> p t b", b=BL
                )[:, 0:w_p, b : b + 1]
                nc.scalar.activation(
                    mview, pf_t[:, 0:w_p], mybir.ActivationFunctionType.Exp,
                    bias=bias_sb, scale=1.0 / WSCALE,
                )

            # ---- phase B: the 1024-step recursion ----
            for t in range(T):
                c, ti = divmod(t, 128)
                p_t = prpsum.tile([NO, BL], F32, tag="pr")
                nc.tensor.matmul(p_t, trAug_sb, s_cur, start=True, stop=True)
                s_next = spool.tile([NS, BL], F32, tag="state")
                nc.vector.tensor_mul(
                    s_next,
                    mchunks[c][:, ti * BL : (ti + 1) * BL],
                    p_t[0:NS, :],
                )
                if (t + 1) % R == 0:
                    e = (t + 1) // R - 1
                    srec = scap_sb[0:1, e * BL : (e + 1) * BL]
                    nc.vector.reciprocal(srec, p_t[NO - 1 : NO, :])
                    bc_t = pbp.tile([K, BL], F32, tag="pb")
                    nc.tensor.matmul(bc_t, ones_r_sb, srec, start=True, stop=True)
                    nc.vector.tensor_mul(s_next[0:K, :], s_next[0:K, :], bc_t)
                s_cur = s_next

            # ---- outputs ----
            nc.sync.dma_start(sfinal[:, :], s_cur)
            nc.sync.dma_start(scap_d[:, :], scap_sb)

    nc.compile()
    return nc


def kernel(hidden, W, b, log_transitions, tags, lengths):
    hidden = np.asarray(hidden, dtype=np.float32)
    W = np.asarray(W, dtype=np.float32)
    b = np.asarray(b, dtype=np.float32)
    trans = np.asarray(log_transitions, dtype=np.float32)
    tags = np.asarray(tags, dtype=np.int32)
    lengths = np.asarray(lengths, dtype=np.int32)

    C = np.float64(np.exp(LOGC))
    expTr = np.exp(trans.astype(np.float64))
    trAug = np.zeros((NS, NO), dtype=np.float64)
    trAug[:K, :K] = expTr.T / C
    trAug[:K, K] = expTr[END, :] / C          # Z capture column
    trAug[K, K + 1] = 1.0                     # A' = A + Z
    trAug[K + 1, K + 1] = 1.0
    trAug[:K, NO - 1] = 1.0 / C               # Shat column (partition 64)
    trAug = trAug.astype(np.float32)

    FLATN = NS * NO + K + 2 + NCHUNK * 128 * BL
    DOFF = NS * NO + K + 2
    cpack_head = np.zeros(DOFF, dtype=np.float32)
    cpack_head[0 : NS * NO] = trAug.reshape(-1)
    cpack_head[NS * NO : NS * NO + K] = b

    # ---- length-ranked round-robin assignment + exact per-slot widths ----
    order = np.argsort(-lengths.astype(np.int64), kind="stable")
    Lsort = lengths.astype(np.int64)[order]
    wvals = tuple(
        min(T, int(-(-Lsort[bslot * NCORE] // 4)) * 4) for bslot in range(BL)
    )
    pieces, hidcol = _pieces(wvals)

    v = (lengths.astype(np.int64) - 1)        # capture step per sequence
    pos = np.arange(T)[None, :]
    maskT = pos < lengths[:, None]
    is_last = pos == (lengths[:, None] - 1)
    emask = (maskT & ~is_last)

    # ---- fp8 packed, transposed hidden ----
    h8 = hidden.astype(ml_dtypes.float8_e4m3)
    wT8 = np.ascontiguousarray(
        (W * WSCALE).astype(ml_dtypes.float8_e4m3)
        .T.reshape(HC, 128, K).transpose(1, 0, 2)
    ).reshape(128, HC * K)

    in_maps = []
    gidx_all = []
    for core in range(NCORE):
        gidx = order[np.arange(BL) * NCORE + core]
        gidx_all.append(gidx)
        hpack = np.empty((128, hidcol + HC * K), dtype=ml_dtypes.float8_e4m3)
        for bslot, c_i, w_p, off in pieces:
            t0 = c_i * 128
            block = h8[gidx[bslot], t0 : t0 + w_p, :].reshape(w_p, HC, 128)
            hpack[:, off : off + HC * w_p] = (
                block.transpose(2, 1, 0).reshape(128, HC * w_p)
            )
        hpack[:, hidcol:] = wT8
        v_c = v[gidx]
        tt = np.arange(T)
        delta = (tt[:, None] == v_c[None, :]).astype(np.float32)   # [T, BL]
        cpack = np.empty((1, FLATN), dtype=np.float32)
        cpack[0, 0:DOFF] = cpack_head
        cpack[0, DOFF:] = delta.reshape(-1)
        in_maps.append({"hpack": hpack, "cpack": cpack})

    if wvals not in _NC_CACHE:
        _NC_CACHE[wvals] = build_bass(wvals)
    nc = _NC_CACHE[wvals]

    res = run_bass_kernel_spmd(nc, in_maps, core_ids=list(range(NCORE)))
    outs = res.results

    # ---- host gold score (exact f32): transitions + emissions ----
    tags_ext = np.concatenate(
        [np.full((B, 1), ROOT, tags.dtype), tags], axis=1
    )
    tr_score = (trans[tags, tags_ext[:, :-1]].astype(np.float64) * maskT).sum(axis=1)
    emit_score = np.zeros(B, dtype=np.float64)
    for core in range(NCORE):
        bs = slice(core * BL, (core + 1) * BL)
        Wg = W[tags[bs]]                                     # [BL, T, H]
        ef = np.einsum("bth,bth->bt", hidden[bs], Wg) + b[tags[bs]]
        emit_score[bs] = (ef.astype(np.float64) * emask[bs]).sum(axis=1)

    # ---- assemble nll ----
    nll = np.zeros(B, dtype=np.float64)
    ev_steps = R * np.arange(1, NEV + 1) - 1                 # [NEV]
    for core in range(NCORE):
        gidx = gidx_all[core]
        v_c = v[gidx]
        sfin = outs[core]["sfinal"].astype(np.float64)
        scap = outs[core]["scap"].reshape(NEV, BL).astype(np.float64)
        AZ = sfin[K] + sfin[K + 1]
        prefix_mask = ev_steps[:, None] < v_c[None, :]
        logS_prefix = (-np.log(scap) * prefix_mask).sum(axis=0)
        log_z = np.log(AZ) + (v_c + 1) * LOGC + logS_prefix
        nll[gidx] = log_z - tr_score[gidx] - emit_score[gidx]

    return nll.astype(np.float32)
